# revision 1
# baseline (speedup 1.0000x reference)
"""HDC Level Encoder kernel for 8 Trainium2 NeuronCores.

Strategy (D=100000 hypervector dim sharded 8 ways, padded to 12800/core):
  - level-table lookups as one-hot matmuls on PE (tables stream once, bf16-exact
    for +-1 entries); x/y/z lookups accumulate the bundle sum directly in PSUM.
  - bind with time hv on DVE, multibind product over the N=128 window via PE
    transpose to d-on-partitions layout + pairwise DVE fold tree.
  - Sinusoid einsum as fp32 PE matmuls against a host-pretransposed [600, D]
    weight stack with a block-diagonal feature matrix.
  - cos(p+b)*sin(p) via ScalarE Sin with explicit range reduction in cycle
    units: m = mod(u,1); Sin(2*pi*m - pi) = -sin(2*pi*u); the two minus signs
    cancel in the product.
  - combine + hard_quantize on DVE, transpose back, DMA out.

Host does only O(N*levels) prep: index math (bit-identical to the reference's
f32 ops), one-hot construction, weight restacking/padding, and sharding.
"""

import sys

for _p in ("/opt/trn_rl_repo",):
    if _p not in sys.path:
        sys.path.insert(0, _p)

import numpy as np

import concourse.bacc as bacc
import concourse.mybir as mybir
import concourse.tile as tile
from concourse import bass_utils, masks

BF16 = mybir.dt.np(mybir.dt.bfloat16)

D = 100000          # true hypervector dim
NCORES = 8
DC = 12800          # per-core padded dim
DP = DC * NCORES    # 102400
N = 128             # window length
LEVELS = 100
TIMESTAMPS = 128
CH = 25             # chunks of 512 per core
CW = 512            # chunk width
NSUB = CH * 4       # 100 sub-chunks of 128
NK = 24             # sinusoid kernels (6 big + 18 small)
KROWS = 600         # stacked contraction dim (6*91 + 18*3)

F32 = mybir.dt.float32
I32 = mybir.dt.int32
BF = mybir.dt.bfloat16
AF = mybir.AluOpType

_TWO_PI = np.float32(2.0 * np.pi)
_PI = np.float32(np.pi)
_INV_2PI = np.float32(1.0 / (2.0 * np.pi))

_nc_cache = {}


def _build_nc():
    nc = bacc.Bacc("TRN2", target_bir_lowering=False, debug=False)

    lvlx = nc.dram_tensor("lvlx", [LEVELS, DC], BF, kind="ExternalInput")
    lvly = nc.dram_tensor("lvly", [LEVELS, DC], BF, kind="ExternalInput")
    lvlz = nc.dram_tensor("lvlz", [LEVELS, DC], BF, kind="ExternalInput")
    lvlt = nc.dram_tensor("lvlt", [TIMESTAMPS, DC], BF, kind="ExternalInput")
    ohx = nc.dram_tensor("ohx", [LEVELS, N], BF, kind="ExternalInput")
    ohy = nc.dram_tensor("ohy", [LEVELS, N], BF, kind="ExternalInput")
    ohz = nc.dram_tensor("ohz", [LEVELS, N], BF, kind="ExternalInput")
    oht = nc.dram_tensor("oht", [TIMESTAMPS, N], BF, kind="ExternalInput")
    wstk = nc.dram_tensor("wstk", [KROWS, DC], F32, kind="ExternalInput")
    fbd = nc.dram_tensor("fbd", [KROWS, NK], F32, kind="ExternalInput")
    bsh = nc.dram_tensor("bsh", [N, NSUB * NK], F32, kind="ExternalInput")
    out = nc.dram_tensor("out", [NSUB, N], F32, kind="ExternalOutput")

    kchunks = [(0, 128), (128, 128), (256, 128), (384, 128), (512, 88)]

    with tile.TileContext(nc) as tc:
        with (
            tc.tile_pool(name="const", bufs=1) as constp,
            tc.tile_pool(name="grand", bufs=1) as grandp,
        ):
            ident_bf = constp.tile([128, 128], BF)
            ident_f32 = constp.tile([128, 128], F32)
            masks.make_identity(nc, ident_bf[:])
            masks.make_identity(nc, ident_f32[:])

            ohx_sb = constp.tile([LEVELS, N], BF, tag="ohx")
            ohy_sb = constp.tile([LEVELS, N], BF, tag="ohy")
            ohz_sb = constp.tile([LEVELS, N], BF, tag="ohz")
            oht_sb = constp.tile([TIMESTAMPS, N], BF, tag="oht")
            nc.sync.dma_start(ohx_sb[:], ohx.ap())
            nc.sync.dma_start(ohy_sb[:], ohy.ap())
            nc.sync.dma_start(ohz_sb[:], ohz.ap())
            nc.sync.dma_start(oht_sb[:], oht.ap())

            fbd_sb = []
            for i, (r0, rn) in enumerate(kchunks):
                t = constp.tile([rn, NK], F32, tag=f"fbd{i}")
                nc.sync.dma_start(t[:], fbd.ap()[r0 : r0 + rn, :])
                fbd_sb.append(t)

            bsh_sb = constp.tile([N, NSUB * NK], F32, tag="bsh")
            nc.sync.dma_start(bsh_sb[:], bsh.ap())

            hvg = grandp.tile([128, NSUB], F32, tag="hvg")
            fg = grandp.tile([128, NSUB * NK], F32, tag="fg")

            # ---------------- phase A: lookups, bind, window product ----------
            with (
                tc.tile_pool(name="tabs", bufs=3) as tabp,
                tc.tile_pool(name="binds", bufs=3) as bindp,
                tc.tile_pool(name="folds", bufs=2) as foldp,
                tc.tile_pool(name="psA", bufs=2, space="PSUM") as psa,
            ):
                for c in range(CH):
                    cs = slice(c * CW, (c + 1) * CW)
                    tx = tabp.tile([LEVELS, CW], BF, tag="tx")
                    ty = tabp.tile([LEVELS, CW], BF, tag="ty")
                    tz = tabp.tile([LEVELS, CW], BF, tag="tz")
                    tt = tabp.tile([TIMESTAMPS, CW], BF, tag="tt")
                    nc.sync.dma_start(tx[:], lvlx.ap()[:, cs])
                    nc.sync.dma_start(ty[:], lvly.ap()[:, cs])
                    nc.sync.dma_start(tz[:], lvlz.ap()[:, cs])
                    nc.sync.dma_start(tt[:], lvlt.ap()[:, cs])

                    ps3 = psa.tile([128, CW], F32, tag="ps3")
                    nc.tensor.matmul(ps3[:], ohx_sb[:], tx[:], start=True, stop=False)
                    nc.tensor.matmul(ps3[:], ohy_sb[:], ty[:], start=False, stop=False)
                    nc.tensor.matmul(ps3[:], ohz_sb[:], tz[:], start=False, stop=True)

                    ptl = psa.tile([128, CW], F32, tag="ptl")
                    nc.tensor.matmul(ptl[:], oht_sb[:], tt[:], start=True, stop=True)

                    tl_sb = bindp.tile([128, CW], BF, tag="tl_sb")
                    nc.scalar.copy(tl_sb[:], ptl[:])
                    bind_sb = bindp.tile([128, CW], BF, tag="bind_sb")
                    nc.vector.tensor_mul(bind_sb[:], ps3[:], tl_sb[:])

                    pbt = psa.tile([128, CW], BF, tag="pbt")
                    for s in range(4):
                        ss = slice(s * 128, (s + 1) * 128)
                        nc.tensor.matmul(
                            pbt[:, ss], bind_sb[:, ss], ident_bf[:], is_transpose=True
                        )
                    bt_sb = bindp.tile([128, CW], BF, tag="bt_sb")
                    nc.scalar.copy(bt_sb[:], pbt[:])

                    # pairwise fold over the window dim (free axis, 4 blocks)
                    src = bt_sb[:].rearrange("p (s n) -> p s n", s=4)
                    w = 64
                    while w >= 1:
                        if w == 1:
                            dst_ap = hvg[:, c * 4 : c * 4 + 4].rearrange(
                                "p (s n) -> p s n", n=1
                            )
                        else:
                            t_new = foldp.tile([128, 4 * w], F32, tag=f"fold{w}")
                            dst_ap = t_new[:].rearrange("p (s n) -> p s n", s=4)
                        nc.vector.tensor_mul(
                            dst_ap, src[:, :, 0:w], src[:, :, w : 2 * w]
                        )
                        if w > 1:
                            src = dst_ap
                        w //= 2

            # ---------------- phase B: sinusoid features ----------------------
            with (
                tc.tile_pool(name="wts", bufs=3) as wp,
                tc.tile_pool(name="trig", bufs=3) as trp,
                tc.tile_pool(name="psB", bufs=2, space="PSUM") as psb,
            ):
                for c in range(CH):
                    cs = slice(c * CW, (c + 1) * CW)
                    wts = []
                    for i, (r0, rn) in enumerate(kchunks):
                        t = wp.tile([rn, CW], F32, tag=f"w{i}")
                        nc.sync.dma_start(t[:], wstk.ap()[r0 : r0 + rn, cs])
                        wts.append(t)

                    ppb = psb.tile([NK, CW], F32, tag="ppb")
                    for i in range(5):
                        nc.tensor.matmul(
                            ppb[:],
                            fbd_sb[i][:],
                            wts[i][:],
                            start=(i == 0),
                            stop=(i == 4),
                        )
                    pbk_sb = trp.tile([NK, CW], F32, tag="pbk_sb")
                    nc.scalar.copy(pbk_sb[:], ppb[:])

                    ppt = psb.tile([128, 4 * NK], F32, tag="ppt")
                    for s in range(4):
                        nc.tensor.matmul(
                            ppt[:, s * NK : (s + 1) * NK],
                            pbk_sb[:, s * 128 : (s + 1) * 128],
                            ident_f32[:NK, :NK],
                            is_transpose=True,
                        )

                    fs = slice(c * 4 * NK, (c + 1) * 4 * NK)
                    # range reduction in cycle units: r = u - rint(u) in
                    # [-0.5, 0.5] (DVE f32->int32 copy rounds half-to-even,
                    # and the subtraction is exact), then Sin(2*pi*r) =
                    # sin(2*pi*u) on ScalarE's [-pi, pi] domain.
                    u = trp.tile([128, 4 * NK], F32, tag="u")
                    nc.vector.tensor_scalar_mul(u[:], ppt[:], float(_INV_2PI))
                    i1 = trp.tile([128, 4 * NK], I32, tag="i1")
                    nc.vector.tensor_copy(i1[:], u[:])
                    m1 = trp.tile([128, 4 * NK], F32, tag="m1")
                    nc.vector.tensor_sub(m1[:], u[:], i1[:])
                    nc.vector.tensor_scalar_mul(m1[:], m1[:], float(_TWO_PI))
                    s1 = trp.tile([128, 4 * NK], F32, tag="s1")
                    nc.scalar.activation(
                        s1[:], m1[:], mybir.ActivationFunctionType.Sin
                    )
                    u2 = trp.tile([128, 4 * NK], F32, tag="u2")
                    nc.vector.tensor_add(u2[:], u[:], bsh_sb[:, fs])
                    i2 = trp.tile([128, 4 * NK], I32, tag="i2")
                    nc.vector.tensor_copy(i2[:], u2[:])
                    m2 = trp.tile([128, 4 * NK], F32, tag="m2")
                    nc.vector.tensor_sub(m2[:], u2[:], i2[:])
                    nc.vector.tensor_scalar_mul(m2[:], m2[:], float(_TWO_PI))
                    s2 = trp.tile([128, 4 * NK], F32, tag="s2")
                    nc.scalar.activation(
                        s2[:], m2[:], mybir.ActivationFunctionType.Sin
                    )
                    nc.vector.tensor_mul(fg[:, fs], s2[:], s1[:])

            # ---------------- combine + hard quantize -------------------------
            with (
                tc.tile_pool(name="comb", bufs=1) as cp,
                tc.tile_pool(name="psC", bufs=1, space="PSUM") as psc,
            ):
                f3 = fg[:].rearrange("p (s k) -> p s k", k=NK)

                def f(k):
                    return f3[:, :, k : k + 1]

                def tmp(tag):
                    return cp.tile([128, NSUB], F32, tag=tag, name=tag)

                a1 = tmp("a1")
                a1v = a1[:].rearrange("p (s k) -> p s k", k=1)
                nc.vector.tensor_add(a1v, f(6), f(21))
                nc.vector.tensor_add(a1v, a1v, f(23))
                q1 = tmp("q1")
                q1v = q1[:].rearrange("p (s k) -> p s k", k=1)
                hvv = hvg[:].rearrange("p (s k) -> p s k", k=1)
                nc.vector.tensor_mul(q1v, hvv, a1v)
                a2 = tmp("a2")
                a2v = a2[:].rearrange("p (s k) -> p s k", k=1)
                nc.vector.tensor_add(a2v, f(9), f(10))
                nc.vector.tensor_mul(q1v, q1v, a2v)
                for k in (11, 12, 17, 18):
                    nc.vector.tensor_mul(q1v, q1v, f(k))

                a3 = tmp("a3")
                a3v = a3[:].rearrange("p (s k) -> p s k", k=1)
                nc.vector.tensor_add(a3v, f(6), f(10))
                nc.vector.tensor_add(a3v, a3v, f(11))
                nc.vector.tensor_add(a3v, a3v, f(12))
                p2 = tmp("p2")
                p2v = p2[:].rearrange("p (s k) -> p s k", k=1)
                nc.vector.tensor_mul(p2v, f(0), f(1))
                for k in (2, 3, 4, 5):
                    nc.vector.tensor_mul(p2v, p2v, f(k))
                q2 = tmp("q2")
                q2v = q2[:].rearrange("p (s k) -> p s k", k=1)
                nc.vector.tensor_mul(q2v, hvv, a3v)
                nc.vector.tensor_mul(q2v, q2v, p2v)

                comb = tmp("comb")
                nc.vector.tensor_add(comb[:], q1[:], q2[:])
                outq = tmp("outq")
                nc.vector.tensor_scalar(outq[:], comb[:], 0.0, 2.0, AF.is_gt, AF.mult)
                nc.vector.tensor_scalar(outq[:], outq[:], -1.0, None, AF.add)

                pso = psc.tile([NSUB, 128], F32, tag="pso")
                nc.tensor.matmul(
                    pso[:], outq[:], ident_f32[:], is_transpose=True
                )
                out_sb = cp.tile([NSUB, 128], F32, tag="out_sb")
                nc.scalar.copy(out_sb[:], pso[:])
                nc.sync.dma_start(out.ap(), out_sb[:])

    nc.compile()
    return nc


def _get_nc():
    if "nc" not in _nc_cache:
        _nc_cache["nc"] = _build_nc()
    return _nc_cache["nc"]


def _value_to_index(x, low, high, num):
    """Bit-identical (f32 elementwise IEEE ops) to the reference's jnp math."""
    x = x.astype(np.float32)
    xc = np.clip(x, np.float32(low), np.float32(high))
    t = (xc - np.float32(low)) / np.float32(high - low) * np.float32(num - 1)
    idx = np.round(t)  # round-half-even, same as jnp.round
    return np.clip(idx, 0, num - 1).astype(np.int32)


def _onehot(idx, levels):
    o = np.zeros((levels, N), dtype=BF16)
    o[idx, np.arange(N)] = 1
    return o


def prepare_in_maps(
    input,
    feat,
    level_x,
    level_y,
    level_z,
    level_t,
    W_big,
    b_big,
    W_small,
    b_small,
):
    ix = _value_to_index(input[:, 1], -5.0, 5.0, LEVELS)
    iy = _value_to_index(input[:, 2], -5.0, 5.0, LEVELS)
    iz = _value_to_index(input[:, 3], -5.0, 5.0, LEVELS)
    it = _value_to_index(input[:, 0], 0.0, float(TIMESTAMPS), TIMESTAMPS)
    ohx = _onehot(ix, LEVELS)
    ohy = _onehot(iy, LEVELS)
    ohz = _onehot(iz, LEVELS)
    oht = _onehot(it, TIMESTAMPS)

    featb = feat[:546].reshape(6, 91).astype(np.float32)
    feats = feat[546:600].reshape(18, 3).astype(np.float32)
    fbd = np.zeros((KROWS, NK), dtype=np.float32)
    for k in range(6):
        fbd[k * 91 : (k + 1) * 91, k] = featb[k]
    for k in range(18):
        fbd[546 + k * 3 : 546 + (k + 1) * 3, 6 + k] = feats[k]

    def padD(a):
        w = [(0, 0)] * a.ndim
        w[-1] = (0, DP - D)
        return np.pad(a, w)

    # tables -> bf16 (exact for +-1), padded
    lx = padD(level_x).astype(BF16)
    ly = padD(level_y).astype(BF16)
    lz = padD(level_z).astype(BF16)
    lt = padD(level_t).astype(BF16)

    # W stack [600, DP] f32: rows = (kernel-major, in-feature) of W_big/W_small
    wb = np.ascontiguousarray(W_big.transpose(0, 2, 1)).reshape(546, D)
    ws = np.ascontiguousarray(W_small.transpose(0, 2, 1)).reshape(54, D)
    wstk = padD(np.concatenate([wb, ws], axis=0)).astype(np.float32)

    # b shift in cycles (+0.25 for the cos->sin shift), d-on-partitions layout
    ball = np.concatenate([b_big, b_small], axis=0).astype(np.float64)
    bsh_full = padD((ball / (2.0 * np.pi) + 0.25).astype(np.float32))  # [24, DP]

    in_maps = []
    for ci in range(NCORES):
        ds = slice(ci * DC, (ci + 1) * DC)
        bs = (
            bsh_full[:, ds]
            .reshape(NK, NSUB, 128)
            .transpose(2, 1, 0)
            .reshape(128, NSUB * NK)
        )
        in_maps.append(
            {
                "lvlx": np.ascontiguousarray(lx[:, ds]),
                "lvly": np.ascontiguousarray(ly[:, ds]),
                "lvlz": np.ascontiguousarray(lz[:, ds]),
                "lvlt": np.ascontiguousarray(lt[:, ds]),
                "ohx": ohx,
                "ohy": ohy,
                "ohz": ohz,
                "oht": oht,
                "wstk": np.ascontiguousarray(wstk[:, ds]),
                "fbd": fbd,
                "bsh": np.ascontiguousarray(bs),
            }
        )
    return in_maps


def kernel(**inputs):
    nc = _get_nc()
    in_maps = prepare_in_maps(**inputs)
    _nc_cache["last_in_maps"] = in_maps
    res = bass_utils.run_bass_kernel_spmd(nc, in_maps, core_ids=list(range(NCORES)))
    shards = [res.results[ci]["out"].reshape(-1) for ci in range(NCORES)]
    return np.concatenate(shards)[:D].astype(np.float32)



# revision 5
# speedup vs baseline: 1.6496x; 1.6496x over previous
"""HDC Level Encoder kernel — single Trainium2 NeuronCore.

Strategy (whole D=100000 on ONE core; launch overhead on the axon-tunneled
8-core path dominates device exec, so fewer cores + denser per-core work wins):
  - level-table lookups as one-hot matmuls on PE with fp8 tables (+-1 exact);
    x/y/z accumulate the bundle sum in PSUM, t looked up separately, bind on
    DVE, PE transpose to d-on-partitions, window product via Pool-engine
    pairwise fold tree.
  - Sinusoid einsum as PE matmuls against a host-pretransposed W stack with a
    block-diagonal feature matrix (feat pre-scaled by 1/2pi so the matmul
    output is already in cycle units).
  - trig via rint-subtract range reduction (DVE) + ScalarE Sin with the 2pi
    scale fused into the activation; cos(p+b) = sin(p + b + pi/2) folded into
    the host-precomputed phase shift.
  - combine + hard_quantize on DVE, signs bit-packed to uint8 on device
    (8x smaller output transfer), unpacked on host.
  - all steady-state DMA is chunk-major contiguous: 2 big DMAs per 512-wide
    chunk (tables 256KB, W-stack+phase-shift 1.33MB).

Host does only O(N*levels + D-byte-shuffling) prep: index math (bit-identical
to the reference's f32 ops), one-hot construction, weight restacking/padding.
"""

import sys

for _p in ("/opt/trn_rl_repo",):
    if _p not in sys.path:
        sys.path.insert(0, _p)

import numpy as np

import concourse.bacc as bacc
import concourse.mybir as mybir
import concourse.tile as tile
from concourse import bass_utils, masks

F32 = mybir.dt.float32
F32R = mybir.dt.float32r
I32 = mybir.dt.int32
BF = mybir.dt.bfloat16
FP8 = mybir.dt.float8e4
U8 = mybir.dt.uint8
AF = mybir.AluOpType

BF16 = mybir.dt.np(BF)
FP8NP = mybir.dt.np(FP8)

D = 100000          # true hypervector dim
NCORES = 1
CW = 512            # chunk width (one PSUM bank of f32)
CH = 196            # chunks per core
DC = CH * CW        # per-core padded dim (100352)
N = 128             # window length
LEVELS = 100
TIMESTAMPS = 128
NSUB = CH * 4       # 128-wide sub-chunks per core
G = NSUB // 8       # packed output bytes per partition
NK = 24             # sinusoid kernels (6 big + 18 small)
KROWS = 600         # stacked contraction dim (6*91 + 18*3)
KBLK = 5            # 128-row blocks of the (padded) contraction dim
WROWS = KBLK * 128  # padded contraction rows (640)
WB_BYTES = KBLK * CW * 4 + 4 * NK * 4  # per-chunk per-partition stream bytes

# F32R: single-pass fp32 matmul (4x faster, slightly different numerics).
# Verified bit-stable on the harness's fixed inputs; F32 is the safe fallback.
WMM = F32

_TWO_PI = float(2.0 * np.pi)

_nc_cache = {}


def _build_nc():
    nc = bacc.Bacc("TRN2", target_bir_lowering=False, debug=False)

    tabs = nc.dram_tensor("tabs", [CH, 128, 4 * CW], FP8, kind="ExternalInput")
    oh = nc.dram_tensor("oh", [128, 4 * 128], FP8, kind="ExternalInput")
    wb = nc.dram_tensor("wb", [CH, 128, WB_BYTES], U8, kind="ExternalInput")
    fbd = nc.dram_tensor("fbd", [WROWS, NK], F32, kind="ExternalInput")
    out = nc.dram_tensor("out", [128, G], U8, kind="ExternalOutput")

    with tile.TileContext(nc) as tc:
        with (
            tc.tile_pool(name="const", bufs=1) as constp,
            tc.tile_pool(name="grand", bufs=1) as grandp,
        ):
            ident_bf = constp.tile([128, 128], BF)
            ident_f32 = constp.tile([128, 128], F32)
            masks.make_identity(nc, ident_bf[:])
            masks.make_identity(nc, ident_f32[:])

            oh_sb = constp.tile([128, 4 * 128], FP8, tag="oh")
            nc.sync.dma_start(oh_sb[:], oh.ap())

            fbd_sb = []
            for j in range(KBLK):
                t = constp.tile([128, NK], WMM, tag=f"fbd{j}")
                nc.sync.dma_start(
                    t[:].bitcast(F32), fbd.ap()[j * 128 : (j + 1) * 128, :]
                )
                fbd_sb.append(t)

            hvg = grandp.tile([128, NSUB], F32, tag="hvg")
            fg = grandp.tile([128, NSUB * NK], F32, tag="fg")

            # ---------------- phase A: lookups, bind, window product ----------
            with (
                tc.tile_pool(name="tabs", bufs=3) as tabp,
                tc.tile_pool(name="binds", bufs=3) as bindp,
                tc.tile_pool(name="folds", bufs=2) as foldp,
                tc.tile_pool(name="psA", bufs=2, space="PSUM") as psa,
            ):
                for c in range(CH):
                    tab = tabp.tile([128, 4 * CW], FP8, tag="tab")
                    nc.sync.dma_start(tab[:], tabs.ap()[c])

                    ps3 = psa.tile([128, CW], F32, tag="ps3")
                    nc.tensor.matmul(
                        ps3[:], oh_sb[:, 0:128], tab[:, 0:CW],
                        start=True, stop=False,
                    )
                    nc.tensor.matmul(
                        ps3[:], oh_sb[:, 128:256], tab[:, CW : 2 * CW],
                        start=False, stop=False,
                    )
                    nc.tensor.matmul(
                        ps3[:], oh_sb[:, 256:384], tab[:, 2 * CW : 3 * CW],
                        start=False, stop=True,
                    )
                    ptl = psa.tile([128, CW], F32, tag="ptl")
                    nc.tensor.matmul(
                        ptl[:], oh_sb[:, 384:512], tab[:, 3 * CW : 4 * CW],
                        start=True, stop=True,
                    )

                    tl_sb = bindp.tile([128, CW], BF, tag="tl_sb")
                    nc.scalar.copy(tl_sb[:], ptl[:])
                    bind_sb = bindp.tile([128, CW], BF, tag="bind_sb")
                    nc.vector.tensor_mul(bind_sb[:], ps3[:], tl_sb[:])

                    pbt = psa.tile([128, CW], BF, tag="pbt")
                    for s in range(4):
                        ss = slice(s * 128, (s + 1) * 128)
                        nc.tensor.matmul(
                            pbt[:, ss], bind_sb[:, ss], ident_bf[:], is_transpose=True
                        )
                    bt_sb = bindp.tile([128, CW], F32, tag="bt_sb")
                    nc.scalar.copy(bt_sb[:], pbt[:])

                    # pairwise fold over the window dim on the Pool engine
                    src = bt_sb[:].rearrange("p (s n) -> p s n", s=4)
                    w = 64
                    while w >= 1:
                        if w == 1:
                            dst_ap = hvg[:, c * 4 : c * 4 + 4].rearrange(
                                "p (s n) -> p s n", n=1
                            )
                        else:
                            t_new = foldp.tile([128, 4 * w], F32, tag=f"fold{w}")
                            dst_ap = t_new[:].rearrange("p (s n) -> p s n", s=4)
                        nc.vector.tensor_mul(
                            dst_ap, src[:, :, 0:w], src[:, :, w : 2 * w]
                        )
                        if w > 1:
                            src = dst_ap
                        w //= 2

            # ---------------- phase B: sinusoid features ----------------------
            with (
                tc.tile_pool(name="wts", bufs=3) as wp,
                tc.tile_pool(name="trig", bufs=3) as trp,
                tc.tile_pool(name="psB", bufs=2, space="PSUM") as psb,
            ):
                for c in range(CH):
                    wbt = wp.tile([128, WB_BYTES], U8, tag="wbt")
                    nc.sync.dma_start(wbt[:], wb.ap()[c])

                    ppb = psb.tile([NK, CW], F32, tag="ppb")
                    for j in range(KBLK):
                        wap = wbt[:, j * CW * 4 : (j + 1) * CW * 4].bitcast(WMM)
                        nc.tensor.matmul(
                            ppb[:], fbd_sb[j][:], wap,
                            start=(j == 0), stop=(j == KBLK - 1),
                        )
                    pbk_sb = trp.tile([NK, CW], F32, tag="pbk_sb")
                    nc.scalar.copy(pbk_sb[:], ppb[:])

                    ppt = psb.tile([128, 4 * NK], F32, tag="ppt")
                    for s in range(4):
                        nc.tensor.matmul(
                            ppt[:, s * NK : (s + 1) * NK],
                            pbk_sb[:, s * 128 : (s + 1) * 128],
                            ident_f32[:NK, :NK],
                            is_transpose=True,
                        )

                    bsh_ap = wbt[:, KBLK * CW * 4 :].bitcast(F32)

                    # range reduction: m = u - rint(u) in [-0.5, 0.5] (exact),
                    # Sin(2*pi*m) == sin(2*pi*u) on ScalarE's [-pi, pi] domain.
                    fs = slice(c * 4 * NK, (c + 1) * 4 * NK)
                    i1 = trp.tile([128, 4 * NK], I32, tag="i1")
                    nc.vector.tensor_copy(i1[:], ppt[:])
                    m1 = trp.tile([128, 4 * NK], F32, tag="m1")
                    nc.vector.tensor_sub(m1[:], ppt[:], i1[:])
                    s1 = trp.tile([128, 4 * NK], F32, tag="s1")
                    nc.scalar.activation(
                        s1[:], m1[:], mybir.ActivationFunctionType.Sin,
                        scale=_TWO_PI,
                    )
                    u2 = trp.tile([128, 4 * NK], F32, tag="u2")
                    nc.vector.tensor_add(u2[:], ppt[:], bsh_ap)
                    i2 = trp.tile([128, 4 * NK], I32, tag="i2")
                    nc.vector.tensor_copy(i2[:], u2[:])
                    m2 = trp.tile([128, 4 * NK], F32, tag="m2")
                    nc.vector.tensor_sub(m2[:], u2[:], i2[:])
                    s2 = trp.tile([128, 4 * NK], F32, tag="s2")
                    nc.scalar.activation(
                        s2[:], m2[:], mybir.ActivationFunctionType.Sin,
                        scale=_TWO_PI,
                    )
                    nc.vector.tensor_mul(fg[:, fs], s2[:], s1[:])

            # ---------------- combine + hard quantize + bit-pack --------------
            with tc.tile_pool(name="comb", bufs=1) as cp:
                f3 = fg[:].rearrange("p (s k) -> p s k", k=NK)

                def f(k):
                    return f3[:, :, k : k + 1]

                def tmp(tag):
                    return cp.tile([128, NSUB], F32, tag=tag, name=tag)

                a1 = tmp("a1")
                a1v = a1[:].rearrange("p (s k) -> p s k", k=1)
                nc.vector.tensor_add(a1v, f(6), f(21))
                nc.vector.tensor_add(a1v, a1v, f(23))
                q1 = tmp("q1")
                q1v = q1[:].rearrange("p (s k) -> p s k", k=1)
                hvv = hvg[:].rearrange("p (s k) -> p s k", k=1)
                nc.vector.tensor_mul(q1v, hvv, a1v)
                a2 = tmp("a2")
                a2v = a2[:].rearrange("p (s k) -> p s k", k=1)
                nc.vector.tensor_add(a2v, f(9), f(10))
                nc.vector.tensor_mul(q1v, q1v, a2v)
                for k in (11, 12, 17, 18):
                    nc.vector.tensor_mul(q1v, q1v, f(k))

                a3 = tmp("a3")
                a3v = a3[:].rearrange("p (s k) -> p s k", k=1)
                nc.vector.tensor_add(a3v, f(6), f(10))
                nc.vector.tensor_add(a3v, a3v, f(11))
                nc.vector.tensor_add(a3v, a3v, f(12))
                p2 = tmp("p2")
                p2v = p2[:].rearrange("p (s k) -> p s k", k=1)
                nc.vector.tensor_mul(p2v, f(0), f(1))
                for k in (2, 3, 4, 5):
                    nc.vector.tensor_mul(p2v, p2v, f(k))
                q2 = tmp("q2")
                q2v = q2[:].rearrange("p (s k) -> p s k", k=1)
                nc.vector.tensor_mul(q2v, hvv, a3v)
                nc.vector.tensor_mul(q2v, q2v, p2v)

                comb = tmp("comb")
                nc.vector.tensor_add(comb[:], q1[:], q2[:])
                bits = tmp("bits")
                nc.vector.tensor_scalar(bits[:], comb[:], 0.0, None, AF.is_gt)

                # little-endian bit-pack along the sub-chunk axis
                bv = bits[:].rearrange("p (g e) -> p g e", e=8)
                l1 = cp.tile([128, G * 4], F32, tag="l1")
                l1v = l1[:].rearrange("p (g e) -> p g e", e=4)
                nc.vector.scalar_tensor_tensor(
                    l1v, bv[:, :, 4:8], 16.0, bv[:, :, 0:4], AF.mult, AF.add
                )
                l2 = cp.tile([128, G * 2], F32, tag="l2")
                l2v = l2[:].rearrange("p (g e) -> p g e", e=2)
                nc.vector.scalar_tensor_tensor(
                    l2v, l1v[:, :, 2:4], 4.0, l1v[:, :, 0:2], AF.mult, AF.add
                )
                l3 = cp.tile([128, G], F32, tag="l3")
                l3v = l3[:].rearrange("p (g e) -> p g e", e=1)
                nc.vector.scalar_tensor_tensor(
                    l3v, l2v[:, :, 1:2], 2.0, l2v[:, :, 0:1], AF.mult, AF.add
                )
                outb = cp.tile([128, G], U8, tag="outb")
                nc.vector.tensor_copy(outb[:], l3[:])
                nc.sync.dma_start(out.ap(), outb[:])

    nc.compile()
    return nc


def _get_nc():
    if "nc" not in _nc_cache:
        _nc_cache["nc"] = _build_nc()
    return _nc_cache["nc"]


def _value_to_index(x, low, high, num):
    """Bit-identical (f32 elementwise IEEE ops) to the reference's jnp math."""
    x = x.astype(np.float32)
    xc = np.clip(x, np.float32(low), np.float32(high))
    t = (xc - np.float32(low)) / np.float32(high - low) * np.float32(num - 1)
    idx = np.round(t)  # round-half-even, same as jnp.round
    return np.clip(idx, 0, num - 1).astype(np.int32)


def _pad_d(a, width):
    w = [(0, 0)] * a.ndim
    w[-1] = (0, width - a.shape[-1])
    return np.pad(a, w)


def prepare_in_maps(
    input,
    feat,
    level_x,
    level_y,
    level_z,
    level_t,
    W_big,
    b_big,
    W_small,
    b_small,
):
    ix = _value_to_index(input[:, 1], -5.0, 5.0, LEVELS)
    iy = _value_to_index(input[:, 2], -5.0, 5.0, LEVELS)
    iz = _value_to_index(input[:, 3], -5.0, 5.0, LEVELS)
    it = _value_to_index(input[:, 0], 0.0, float(TIMESTAMPS), TIMESTAMPS)

    oh_pack = np.zeros((128, 4, 128), dtype=FP8NP)
    for i, idx in enumerate((ix, iy, iz, it)):
        oh_pack[idx, i, np.arange(N)] = 1
    oh_pack = oh_pack.reshape(128, 512)

    # feat block-diagonal, pre-scaled by 1/2pi (matmul output in cycle units)
    featb = feat[:546].reshape(6, 91).astype(np.float32)
    feats = feat[546:600].reshape(18, 3).astype(np.float32)
    fbd = np.zeros((WROWS, NK), dtype=np.float32)
    for k in range(6):
        fbd[k * 91 : (k + 1) * 91, k] = featb[k]
    for k in range(18):
        fbd[546 + k * 3 : 546 + (k + 1) * 3, 6 + k] = feats[k]
    fbd *= np.float32(1.0 / _TWO_PI)

    # tables, fp8 (+-1 exact), chunk-major packed: tabs[c, p, i*CW + w]
    tabs = np.zeros((CH, 128, 4, CW), dtype=FP8NP)
    for i, tbl in enumerate((level_x, level_y, level_z, level_t)):
        rows = tbl.shape[0]
        arr = _pad_d(tbl.astype(np.float32), DC).reshape(rows, CH, CW)
        tabs[:, :rows, i, :] = arr.astype(FP8NP).transpose(1, 0, 2)
    tabs = tabs.reshape(CH, 128, 4 * CW)

    # W stack rows = (kernel-major, in-feature) of W_big/W_small, padded to
    # 640 rows x DC cols; chunk-major packed so each chunk is one contiguous
    # [128, KBLK*CW] f32 block; phase shift (cycles, +0.25 for cos->sin)
    # appended per chunk in d-on-partitions layout.
    wbig = np.ascontiguousarray(W_big.transpose(0, 2, 1)).reshape(546, D)
    wsml = np.ascontiguousarray(W_small.transpose(0, 2, 1)).reshape(54, D)
    wrows = np.zeros((WROWS, DC), dtype=np.float32)
    wrows[:KROWS] = _pad_d(
        np.concatenate([wbig, wsml], axis=0).astype(np.float32), DC
    )
    wpart = np.ascontiguousarray(
        wrows.reshape(KBLK, 128, CH, CW).transpose(2, 1, 0, 3)
    )  # [CH, 128, KBLK, CW] f32
    wbytes = wpart.view(np.uint8).reshape(CH, 128, KBLK * CW * 4)

    ball = np.concatenate([b_big, b_small], axis=0).astype(np.float64)
    bsh = _pad_d((ball / (2.0 * np.pi) + 0.25).astype(np.float32), DC)  # [24, DC]
    bpart = np.ascontiguousarray(
        bsh.reshape(NK, CH, 4, 128).transpose(1, 3, 2, 0)
    )  # [CH, 128, 4, NK] f32
    bbytes = bpart.view(np.uint8).reshape(CH, 128, 4 * NK * 4)

    wb = np.ascontiguousarray(np.concatenate([wbytes, bbytes], axis=2))

    return [{"tabs": tabs, "oh": oh_pack, "wb": wb, "fbd": fbd}]


def kernel(**inputs):
    nc = _get_nc()
    in_maps = prepare_in_maps(**inputs)
    _nc_cache["last_in_maps"] = in_maps
    res = bass_utils.run_bass_kernel_spmd(nc, in_maps, core_ids=list(range(NCORES)))
    by = res.results[0]["out"]  # [128, G] uint8
    bits = np.unpackbits(by, axis=1, bitorder="little")  # [128, NSUB]
    vals = bits.astype(np.float32) * 2.0 - 1.0
    return vals.T.reshape(-1)[:D]


# revision 7
# speedup vs baseline: 3.7875x; 2.2960x over previous
"""HDC Level Encoder kernel — single Trainium2 NeuronCore.

Strategy (whole D=100000 on ONE core; launch overhead on the axon-tunneled
8-core path dominates device exec, so fewer cores + denser per-core work wins):
  - level-table lookups as one-hot matmuls on PE with fp8 tables (+-1 exact);
    x/y/z accumulate the bundle sum in PSUM, t looked up separately, bind on
    DVE, PE transpose to d-on-partitions, window product via Pool-engine
    pairwise fold tree.
  - Sinusoid einsum as PE matmuls against a host-pretransposed W stack with a
    block-diagonal feature matrix (feat pre-scaled by 1/2pi so the matmul
    output is already in cycle units).
  - trig via rint-subtract range reduction (DVE) + ScalarE Sin with the 2pi
    scale fused into the activation; cos(p+b) = sin(p + b + pi/2) folded into
    the host-precomputed phase shift.
  - combine + hard_quantize on DVE, signs bit-packed to uint8 on device
    (8x smaller output transfer), unpacked on host.
  - all steady-state DMA is chunk-major contiguous: 2 big DMAs per 512-wide
    chunk (tables 256KB, W-stack+phase-shift 1.33MB).

Host does only O(N*levels + D-byte-shuffling) prep: index math (bit-identical
to the reference's f32 ops), one-hot construction, weight restacking/padding.
"""

import sys

for _p in ("/opt/trn_rl_repo",):
    if _p not in sys.path:
        sys.path.insert(0, _p)

import numpy as np

import concourse.bacc as bacc
import concourse.mybir as mybir
import concourse.tile as tile
from concourse import bass_utils, masks

F32 = mybir.dt.float32
F32R = mybir.dt.float32r
I32 = mybir.dt.int32
BF = mybir.dt.bfloat16
FP8 = mybir.dt.float8e4
U8 = mybir.dt.uint8
AF = mybir.AluOpType

BF16 = mybir.dt.np(BF)
FP8NP = mybir.dt.np(FP8)

D = 100000          # true hypervector dim
NCORES = 1
CW = 512            # chunk width (one PSUM bank of f32)
CH = 196            # chunks per core
DC = CH * CW        # per-core padded dim (100352)
N = 128             # window length
LEVELS = 100
TIMESTAMPS = 128
NSUB = CH * 4       # 128-wide sub-chunks per core
G = NSUB // 8       # packed output bytes per partition
NK = 24             # sinusoid kernels (6 big + 18 small)
KROWS = 600         # stacked contraction dim (6*91 + 18*3)
KBLK = 5            # 128-row blocks of the (padded) contraction dim
WROWS = KBLK * 128  # padded contraction rows (640)
WB_BYTES = KBLK * CW * 4 + 4 * NK * 4  # per-chunk per-partition stream bytes

# F32R: single-pass fp32 matmul (4x faster, slightly different numerics).
# Verified bit-stable on the harness's fixed inputs; F32 is the safe fallback.
WMM = F32

_TWO_PI = float(2.0 * np.pi)

_nc_cache = {}


def _build_nc():
    nc = bacc.Bacc("TRN2", target_bir_lowering=False, debug=False)

    tabs = nc.dram_tensor("tabs", [CH, 128, 4 * CW], FP8, kind="ExternalInput")
    oh = nc.dram_tensor("oh", [128, 4 * 128], FP8, kind="ExternalInput")
    wb = nc.dram_tensor("wb", [CH, 128, WB_BYTES], U8, kind="ExternalInput")
    fbd = nc.dram_tensor("fbd", [WROWS, NK], F32, kind="ExternalInput")
    out = nc.dram_tensor("out", [128, G], U8, kind="ExternalOutput")

    with tile.TileContext(nc) as tc:
        with (
            tc.tile_pool(name="const", bufs=1) as constp,
            tc.tile_pool(name="grand", bufs=1) as grandp,
        ):
            ident_bf = constp.tile([128, 128], BF)
            ident_f32 = constp.tile([128, 128], F32)
            masks.make_identity(nc, ident_bf[:])
            masks.make_identity(nc, ident_f32[:])

            oh_sb = constp.tile([128, 4 * 128], FP8, tag="oh")
            nc.sync.dma_start(oh_sb[:], oh.ap())

            fbd_sb = []
            for j in range(KBLK):
                t = constp.tile([128, NK], WMM, tag=f"fbd{j}")
                nc.sync.dma_start(
                    t[:].bitcast(F32), fbd.ap()[j * 128 : (j + 1) * 128, :]
                )
                fbd_sb.append(t)

            hvg = grandp.tile([128, NSUB], F32, tag="hvg")
            fg = grandp.tile([128, NSUB * NK], F32, tag="fg")

            # ---------------- phase A: lookups, bind, window product ----------
            with (
                tc.tile_pool(name="tabs", bufs=3) as tabp,
                tc.tile_pool(name="binds", bufs=3) as bindp,
                tc.tile_pool(name="folds", bufs=2) as foldp,
                tc.tile_pool(name="psA", bufs=2, space="PSUM") as psa,
            ):
                for c in range(CH):
                    tab = tabp.tile([128, 4 * CW], FP8, tag="tab")
                    nc.sync.dma_start(tab[:], tabs.ap()[c])

                    ps3 = psa.tile([128, CW], F32, tag="ps3")
                    nc.tensor.matmul(
                        ps3[:], oh_sb[:, 0:128], tab[:, 0:CW],
                        start=True, stop=False,
                    )
                    nc.tensor.matmul(
                        ps3[:], oh_sb[:, 128:256], tab[:, CW : 2 * CW],
                        start=False, stop=False,
                    )
                    nc.tensor.matmul(
                        ps3[:], oh_sb[:, 256:384], tab[:, 2 * CW : 3 * CW],
                        start=False, stop=True,
                    )
                    ptl = psa.tile([128, CW], F32, tag="ptl")
                    nc.tensor.matmul(
                        ptl[:], oh_sb[:, 384:512], tab[:, 3 * CW : 4 * CW],
                        start=True, stop=True,
                    )

                    tl_sb = bindp.tile([128, CW], BF, tag="tl_sb")
                    nc.scalar.copy(tl_sb[:], ptl[:])
                    bind_sb = bindp.tile([128, CW], BF, tag="bind_sb")
                    nc.vector.tensor_mul(bind_sb[:], ps3[:], tl_sb[:])

                    pbt = psa.tile([128, CW], BF, tag="pbt")
                    for s in range(4):
                        ss = slice(s * 128, (s + 1) * 128)
                        nc.tensor.matmul(
                            pbt[:, ss], bind_sb[:, ss], ident_bf[:], is_transpose=True
                        )
                    bt_sb = bindp.tile([128, CW], F32, tag="bt_sb")
                    nc.scalar.copy(bt_sb[:], pbt[:])

                    # pairwise fold over the window dim on the Pool engine
                    src = bt_sb[:].rearrange("p (s n) -> p s n", s=4)
                    w = 64
                    while w >= 1:
                        if w == 1:
                            dst_ap = hvg[:, c * 4 : c * 4 + 4].rearrange(
                                "p (s n) -> p s n", n=1
                            )
                        else:
                            t_new = foldp.tile([128, 4 * w], F32, tag=f"fold{w}")
                            dst_ap = t_new[:].rearrange("p (s n) -> p s n", s=4)
                        nc.vector.tensor_mul(
                            dst_ap, src[:, :, 0:w], src[:, :, w : 2 * w]
                        )
                        if w > 1:
                            src = dst_ap
                        w //= 2

            # ---------------- phase B: sinusoid features ----------------------
            with (
                tc.tile_pool(name="wts", bufs=3) as wp,
                tc.tile_pool(name="trig", bufs=3) as trp,
                tc.tile_pool(name="psB", bufs=2, space="PSUM") as psb,
            ):
                for c in range(CH):
                    wbt = wp.tile([128, WB_BYTES], U8, tag="wbt")
                    nc.sync.dma_start(wbt[:], wb.ap()[c])

                    ppb = psb.tile([NK, CW], F32, tag="ppb")
                    for j in range(KBLK):
                        wap = wbt[:, j * CW * 4 : (j + 1) * CW * 4].bitcast(WMM)
                        nc.tensor.matmul(
                            ppb[:], fbd_sb[j][:], wap,
                            start=(j == 0), stop=(j == KBLK - 1),
                        )
                    pbk_sb = trp.tile([NK, CW], F32, tag="pbk_sb")
                    nc.scalar.copy(pbk_sb[:], ppb[:])

                    ppt = psb.tile([128, 4 * NK], F32, tag="ppt")
                    for s in range(4):
                        nc.tensor.matmul(
                            ppt[:, s * NK : (s + 1) * NK],
                            pbk_sb[:, s * 128 : (s + 1) * 128],
                            ident_f32[:NK, :NK],
                            is_transpose=True,
                        )

                    bsh_ap = wbt[:, KBLK * CW * 4 :].bitcast(F32)

                    # range reduction: m = u - rint(u) in [-0.5, 0.5] (exact),
                    # Sin(2*pi*m) == sin(2*pi*u) on ScalarE's [-pi, pi] domain.
                    fs = slice(c * 4 * NK, (c + 1) * 4 * NK)
                    i1 = trp.tile([128, 4 * NK], I32, tag="i1")
                    nc.vector.tensor_copy(i1[:], ppt[:])
                    m1 = trp.tile([128, 4 * NK], F32, tag="m1")
                    nc.vector.tensor_sub(m1[:], ppt[:], i1[:])
                    s1 = trp.tile([128, 4 * NK], F32, tag="s1")
                    nc.scalar.activation(
                        s1[:], m1[:], mybir.ActivationFunctionType.Sin,
                        scale=_TWO_PI,
                    )
                    u2 = trp.tile([128, 4 * NK], F32, tag="u2")
                    nc.vector.tensor_add(u2[:], ppt[:], bsh_ap)
                    i2 = trp.tile([128, 4 * NK], I32, tag="i2")
                    nc.vector.tensor_copy(i2[:], u2[:])
                    m2 = trp.tile([128, 4 * NK], F32, tag="m2")
                    nc.vector.tensor_sub(m2[:], u2[:], i2[:])
                    s2 = trp.tile([128, 4 * NK], F32, tag="s2")
                    nc.scalar.activation(
                        s2[:], m2[:], mybir.ActivationFunctionType.Sin,
                        scale=_TWO_PI,
                    )
                    nc.vector.tensor_mul(fg[:, fs], s2[:], s1[:])

            # ---------------- combine + hard quantize + bit-pack --------------
            with tc.tile_pool(name="comb", bufs=1) as cp:
                f3 = fg[:].rearrange("p (s k) -> p s k", k=NK)

                def f(k):
                    return f3[:, :, k : k + 1]

                def tmp(tag):
                    return cp.tile([128, NSUB], F32, tag=tag, name=tag)

                a1 = tmp("a1")
                a1v = a1[:].rearrange("p (s k) -> p s k", k=1)
                nc.vector.tensor_add(a1v, f(6), f(21))
                nc.vector.tensor_add(a1v, a1v, f(23))
                q1 = tmp("q1")
                q1v = q1[:].rearrange("p (s k) -> p s k", k=1)
                hvv = hvg[:].rearrange("p (s k) -> p s k", k=1)
                nc.vector.tensor_mul(q1v, hvv, a1v)
                a2 = tmp("a2")
                a2v = a2[:].rearrange("p (s k) -> p s k", k=1)
                nc.vector.tensor_add(a2v, f(9), f(10))
                nc.vector.tensor_mul(q1v, q1v, a2v)
                for k in (11, 12, 17, 18):
                    nc.vector.tensor_mul(q1v, q1v, f(k))

                a3 = tmp("a3")
                a3v = a3[:].rearrange("p (s k) -> p s k", k=1)
                nc.vector.tensor_add(a3v, f(6), f(10))
                nc.vector.tensor_add(a3v, a3v, f(11))
                nc.vector.tensor_add(a3v, a3v, f(12))
                p2 = tmp("p2")
                p2v = p2[:].rearrange("p (s k) -> p s k", k=1)
                nc.vector.tensor_mul(p2v, f(0), f(1))
                for k in (2, 3, 4, 5):
                    nc.vector.tensor_mul(p2v, p2v, f(k))
                q2 = tmp("q2")
                q2v = q2[:].rearrange("p (s k) -> p s k", k=1)
                nc.vector.tensor_mul(q2v, hvv, a3v)
                nc.vector.tensor_mul(q2v, q2v, p2v)

                comb = tmp("comb")
                nc.vector.tensor_add(comb[:], q1[:], q2[:])
                bits = tmp("bits")
                nc.vector.tensor_scalar(bits[:], comb[:], 0.0, None, AF.is_gt)

                # little-endian bit-pack along the sub-chunk axis
                bv = bits[:].rearrange("p (g e) -> p g e", e=8)
                l1 = cp.tile([128, G * 4], F32, tag="l1")
                l1v = l1[:].rearrange("p (g e) -> p g e", e=4)
                nc.vector.scalar_tensor_tensor(
                    l1v, bv[:, :, 4:8], 16.0, bv[:, :, 0:4], AF.mult, AF.add
                )
                l2 = cp.tile([128, G * 2], F32, tag="l2")
                l2v = l2[:].rearrange("p (g e) -> p g e", e=2)
                nc.vector.scalar_tensor_tensor(
                    l2v, l1v[:, :, 2:4], 4.0, l1v[:, :, 0:2], AF.mult, AF.add
                )
                l3 = cp.tile([128, G], F32, tag="l3")
                l3v = l3[:].rearrange("p (g e) -> p g e", e=1)
                nc.vector.scalar_tensor_tensor(
                    l3v, l2v[:, :, 1:2], 2.0, l2v[:, :, 0:1], AF.mult, AF.add
                )
                outb = cp.tile([128, G], U8, tag="outb")
                nc.vector.tensor_copy(outb[:], l3[:])
                nc.sync.dma_start(out.ap(), outb[:])

    nc.compile()
    return nc


def _get_nc():
    if "nc" not in _nc_cache:
        _nc_cache["nc"] = _build_nc()
    return _nc_cache["nc"]


def _value_to_index(x, low, high, num):
    """Bit-identical (f32 elementwise IEEE ops) to the reference's jnp math."""
    x = x.astype(np.float32)
    xc = np.clip(x, np.float32(low), np.float32(high))
    t = (xc - np.float32(low)) / np.float32(high - low) * np.float32(num - 1)
    idx = np.round(t)  # round-half-even, same as jnp.round
    return np.clip(idx, 0, num - 1).astype(np.int32)


def _pad_d(a, width):
    w = [(0, 0)] * a.ndim
    w[-1] = (0, width - a.shape[-1])
    return np.pad(a, w)


def prepare_in_maps(
    input,
    feat,
    level_x,
    level_y,
    level_z,
    level_t,
    W_big,
    b_big,
    W_small,
    b_small,
):
    ix = _value_to_index(input[:, 1], -5.0, 5.0, LEVELS)
    iy = _value_to_index(input[:, 2], -5.0, 5.0, LEVELS)
    iz = _value_to_index(input[:, 3], -5.0, 5.0, LEVELS)
    it = _value_to_index(input[:, 0], 0.0, float(TIMESTAMPS), TIMESTAMPS)

    oh_pack = np.zeros((128, 4, 128), dtype=FP8NP)
    for i, idx in enumerate((ix, iy, iz, it)):
        oh_pack[idx, i, np.arange(N)] = 1
    oh_pack = oh_pack.reshape(128, 512)

    # feat block-diagonal, pre-scaled by 1/2pi (matmul output in cycle units)
    featb = feat[:546].reshape(6, 91).astype(np.float32)
    feats = feat[546:600].reshape(18, 3).astype(np.float32)
    fbd = np.zeros((WROWS, NK), dtype=np.float32)
    for k in range(6):
        fbd[k * 91 : (k + 1) * 91, k] = featb[k]
    for k in range(18):
        fbd[546 + k * 3 : 546 + (k + 1) * 3, 6 + k] = feats[k]
    fbd *= np.float32(1.0 / _TWO_PI)

    # tables, fp8 (+-1 exact), chunk-major packed: tabs[c, p, i*CW + w]
    tabs = np.zeros((CH, 128, 4, CW), dtype=FP8NP)
    for i, tbl in enumerate((level_x, level_y, level_z, level_t)):
        rows = tbl.shape[0]
        arr = _pad_d(tbl.astype(np.float32), DC).reshape(rows, CH, CW)
        tabs[:, :rows, i, :] = arr.astype(FP8NP).transpose(1, 0, 2)
    tabs = tabs.reshape(CH, 128, 4 * CW)

    # W stack rows = (kernel-major, in-feature) of W_big/W_small, padded to
    # 640 rows x DC cols; chunk-major packed so each chunk is one contiguous
    # [128, KBLK*CW] f32 block; phase shift (cycles, +0.25 for cos->sin)
    # appended per chunk in d-on-partitions layout.
    wbig = np.ascontiguousarray(W_big.transpose(0, 2, 1)).reshape(546, D)
    wsml = np.ascontiguousarray(W_small.transpose(0, 2, 1)).reshape(54, D)
    wrows = np.zeros((WROWS, DC), dtype=np.float32)
    wrows[:KROWS] = _pad_d(
        np.concatenate([wbig, wsml], axis=0).astype(np.float32), DC
    )
    wpart = np.ascontiguousarray(
        wrows.reshape(KBLK, 128, CH, CW).transpose(2, 1, 0, 3)
    )  # [CH, 128, KBLK, CW] f32
    wbytes = wpart.view(np.uint8).reshape(CH, 128, KBLK * CW * 4)

    ball = np.concatenate([b_big, b_small], axis=0).astype(np.float64)
    bsh = _pad_d((ball / (2.0 * np.pi) + 0.25).astype(np.float32), DC)  # [24, DC]
    bpart = np.ascontiguousarray(
        bsh.reshape(NK, CH, 4, 128).transpose(1, 3, 2, 0)
    )  # [CH, 128, 4, NK] f32
    bbytes = bpart.view(np.uint8).reshape(CH, 128, 4 * NK * 4)

    wb = np.ascontiguousarray(np.concatenate([wbytes, bbytes], axis=2))

    return [{"tabs": tabs, "oh": oh_pack, "wb": wb, "fbd": fbd}]


def kernel(**inputs):
    nc = _get_nc()
    in_maps = prepare_in_maps(**inputs)
    _nc_cache["last_in_maps"] = in_maps
    res = bass_utils.run_bass_kernel_spmd(nc, in_maps, core_ids=list(range(NCORES)))
    by = res.results[0]["out"]  # [128, G] uint8
    bits = np.unpackbits(by, axis=1, bitorder="little")  # [128, NSUB]
    vals = bits.astype(np.float32) * 2.0 - 1.0
    return vals.T.reshape(-1)[:D]
